# revision 7
# baseline (speedup 1.0000x reference)
"""Trainium2 Bass kernel for the PCNN recurrence (nn_CCNN1d).

Model (per sample, recurrence over T steps, state vectors of length L):
    f = df*f + x_t + conv3(y, w)          # learned 3-tap conv, zero pad
    l = dl*l + (y shifted left + right)   # fixed [1,0,1] kernel
    u = f * (1 + 0.5*l)
    e = de*e + 10*y
    y = sigmoid(u - e)
outputs y per step.

Sharding: data-parallel over batch B=32 -> 4 samples per NeuronCore x 8.

Per-core layout ("fine-L"): L=8192 split into 64 blocks of 128; partition
p = position within block.  Tiles are [128 x 264]; the matmul data window
is columns [2:262) = 4 sample groups of 65 (64 data blocks + 1 zero pad
column); column 1 is the zero left-halo source of sample 0.  The 3-tap
conv along L becomes one banded 128x128 stationary matmul (within-block
taps) plus two single-element "halo" stationaries applied to rhs views
shifted by one column (cross-block taps); the zero pad columns make
sample boundaries behave like zero padding.  The fp32r (tfloat32) matmul
ISA requires the *dst* AP to be 8-byte aligned with even count, which the
[2:262) window satisfies; rhs offsets are unconstrained so the +-1 column
halo shifts ride on the rhs side.

Per step:
    PE   : Pf(psum) = I@x + Wc@y + Hdn@y(<<1) + Hup@y(>>1)
           Pl(psum) = Ddl@l2 + Wl05@y + Hdn05@y(<<1) + Hup05@y(>>1)
    DVE  : f   = df*f + Pf          (scalar_tensor_tensor)
           u   = (Pl + 1) * f
           e2  = de*e2 + y          (e2 = e/10)
           v   = -10*e2 + u
    ACT  : l2  = copy(Pl)           (next step's Ddl operand)
           y   = sigmoid(v)         (strided write; pads stay 0)

conv_mode="tf32_split": the conv matmuls run as fp32r (tfloat32) pairs
(W = Wh + Wl, y = yh + yl, keeping Wh@yh + Wh@yl + Wl@yh), ~4x faster on
the PE than plain fp32 matmul and equal to fp32 to ~1e-4 absmax here.
conv_mode="fp32": plain fp32 matmuls everywhere (slower, exact).
"""

import numpy as np

B, T, L = 32, 64, 8192
N_CORES = 8
BPC = B // N_CORES          # samples per core
NBLK = L // 128             # 64 blocks per sample
GW = NBLK + 1               # sample group width incl. 1 pad col
DO = 2                      # data window offset (8-byte aligned)
DW = BPC * GW               # data window width = 260
TW = DO + DW + 2            # tile width = 264
ALPHA_F, ALPHA_L, ALPHA_E, V_E = 0.1, 1.0, 1.0, 10.0

_CACHE = {}


def _round_tf32(a):
    a = np.asarray(a, np.float32)
    ai = a.view(np.int32).astype(np.int64)
    return (((ai + 0x1000) & ~0x1FFF).astype(np.int32)).view(np.float32).reshape(a.shape)


def _patch_tile_drain():
    """This toolchain's walrus allows at most one sync wait per instruction;
    spread the TileContext final-drain waits over single-wait nops."""
    import concourse.tile as tile
    from concourse.vector_clock import ScopedClock

    if getattr(tile.TileContext, "_drain_patched", False):
        return

    def _drain_and_barrier(self, tick_clock, wait_clock):
        nc = self.nc
        probe = nc.sync.nop()
        wait_clock.add_sem_waits(probe.ins, ScopedClock({None: tick_clock.global_clock}))
        si = probe.ins.sync_info
        waits = list(si.on_wait) if si and si.on_wait else []
        if len(waits) > 1:
            si.on_wait = waits[:1]
            for w in waits[1:]:
                extra = nc.sync.nop()
                esi = extra.ins.sync_info
                if esi is None:
                    from concourse import mybir
                    extra.ins.sync_info = mybir.SyncInfo(on_wait=[w], on_update=[])
                else:
                    esi.on_wait = [w]
        nc.sync.drain()
        nc.all_engine_barrier()
        assert self.sems is not None
        popped = nc._tile_sem_poison_stack.pop()
        assert popped is self._sem_poison
        nc.clear_and_free_semaphores(list(self.sems.allocated().values()))
        nc.all_engine_barrier()

    tile.TileContext._drain_and_barrier = _drain_and_barrier
    tile.TileContext._drain_patched = True


def _split_sync_waits(nc):
    """Hoist extra sync waits (>1 per instruction) onto same-engine nops
    inserted right before the instruction."""
    from concourse import mybir

    ctr = 0
    for f in nc.m.functions:
        for bb in f.blocks:
            insts = list(bb.instructions)
            if not any(i.sync_info and i.sync_info.on_wait
                       and len(i.sync_info.on_wait) > 1 for i in insts):
                continue
            new_insts = []
            for inst in insts:
                si = inst.sync_info
                waits = list(si.on_wait) if si and si.on_wait else []
                if len(waits) > 1:
                    for w in waits[:-1]:
                        nop = mybir.InstNoOp(name=f"I-wsplit{ctr}", ins=[],
                                             outs=[])
                        ctr += 1
                        nop.engine = inst.engine
                        nop.sync_info = mybir.SyncInfo(on_wait=[w],
                                                       on_update=[])
                        new_insts.append(nop)
                    si.on_wait = [waits[-1]]
                new_insts.append(inst)
            try:
                bb.instructions[:] = new_insts
            except TypeError:
                bb.instructions = new_insts


def _build_program(n_steps, conv_mode):
    """Build the Bass module. Returns (nc, input_names)."""
    _patch_tile_drain()
    from contextlib import ExitStack
    import concourse.bass as bass
    import concourse.tile as tile
    from concourse import mybir

    dt = mybir.dt
    AF = mybir.ActivationFunctionType
    OP = mybir.AluOpType
    df = float(np.float32(np.exp(-ALPHA_F)))
    de = float(np.float32(np.exp(-ALPHA_E)))

    nc = bass.Bass("TRN2", target_bir_lowering=False, debug=False,
                   num_devices=N_CORES)

    xp = nc.dram_tensor("xp", [n_steps, 128, TW], dt.float32,
                        kind="ExternalInput").ap()
    stat_names = ["Wc", "Hdn", "Hup", "Wl05", "Hdn05", "Hup05", "Ddl", "Ident"]
    if conv_mode == "tf32_split":
        stat_names += ["Wc_l", "Hdn_l", "Hup_l"]
    stats_dram = {n: nc.dram_tensor(n, [128, 128], dt.float32,
                                    kind="ExternalInput").ap()
                  for n in stat_names}
    yp = nc.dram_tensor("yp", [n_steps, 128, TW], dt.float32,
                        kind="ExternalOutput").ap()

    W = slice(DO, DO + DW)           # data window [2:262)
    WL = slice(DO - 1, DO + DW - 1)  # rhs shifted left  [1:261)
    WR = slice(DO + 1, DO + DW + 1)  # rhs shifted right [3:263)

    with tile.TileContext(nc) as tc:
        with ExitStack() as ctx:
            const = ctx.enter_context(tc.tile_pool(name="const", bufs=1))
            state = ctx.enter_context(tc.tile_pool(name="state", bufs=2))
            ybufs = ctx.enter_context(tc.tile_pool(name="ybufs", bufs=3))
            xbufs = ctx.enter_context(tc.tile_pool(name="xbufs", bufs=3))
            tmp = ctx.enter_context(tc.tile_pool(name="tmp", bufs=2))
            psum = ctx.enter_context(tc.tile_pool(name="psum", bufs=2,
                                                  space="PSUM"))

            stats = {}
            for n in stat_names:
                st = const.tile([128, 128], dt.float32, tag=f"st_{n}")
                nc.sync.dma_start(st[:], stats_dram[n][:])
                stats[n] = st

            stats_r = {}
            if conv_mode == "tf32_split":
                # fp32r matmul operands need an on-chip rounding producer
                for n in ["Wc", "Hdn", "Hup", "Wl05", "Hdn05", "Hup05",
                          "Wc_l", "Hdn_l", "Hup_l"]:
                    sr = const.tile([128, 128], dt.float32r, tag=f"str_{n}")
                    nc.vector.tensor_copy(sr[:], stats[n][:])
                    stats_r[n] = sr

            def S(n):
                return stats[n][:]

            f = state.tile([128, TW], dt.float32, tag="f")
            l2 = state.tile([128, TW], dt.float32, tag="l2")
            e2 = state.tile([128, TW], dt.float32, tag="e2")
            y = ybufs.tile([128, TW], dt.float32, tag="y")
            nc.vector.memset(f[:], 0.0)
            nc.vector.memset(l2[:], 0.0)
            nc.vector.memset(e2[:], 1.0)
            nc.vector.memset(y[:], 0.0)
            if conv_mode == "tf32_split":
                yh = ybufs.tile([128, TW], dt.float32r, tag="yh")
                yl = ybufs.tile([128, TW], dt.float32r, tag="yl")
                nc.vector.memset(yh[:].bitcast(dt.float32), 0.0)
                nc.vector.memset(yl[:].bitcast(dt.float32), 0.0)

            for t in range(n_steps):
                xt = xbufs.tile([128, TW], dt.float32, tag="x")
                nc.sync.dma_start(xt[:], xp[t])

                Pf = psum.tile([128, TW], dt.float32, tag="Pf")
                Pl = psum.tile([128, TW], dt.float32, tag="Pl")
                mm = nc.tensor.matmul
                # ---- Pf = x + conv3(y, w) ----
                mm(Pf[:, W], S("Ident"), xt[:, W], start=True, stop=False)
                if conv_mode == "fp32":
                    mm(Pf[:, W], S("Wc"), y[:, W], start=False, stop=False)
                    mm(Pf[:, W], S("Hdn"), y[:, WL], start=False, stop=False)
                    mm(Pf[:, W], S("Hup"), y[:, WR], start=False, stop=True)
                else:
                    r = stats_r
                    mm(Pf[:, W], r["Wc"][:], yh[:, W], start=False, stop=False)
                    mm(Pf[:, W], r["Wc"][:], yl[:, W], start=False, stop=False)
                    mm(Pf[:, W], r["Wc_l"][:], yh[:, W], start=False, stop=False)
                    mm(Pf[:, W], r["Hdn"][:], yh[:, WL], start=False, stop=False)
                    mm(Pf[:, W], r["Hdn"][:], yl[:, WL], start=False, stop=False)
                    mm(Pf[:, W], r["Hdn_l"][:], yh[:, WL], start=False, stop=False)
                    mm(Pf[:, W], r["Hup"][:], yh[:, WR], start=False, stop=False)
                    mm(Pf[:, W], r["Hup"][:], yl[:, WR], start=False, stop=False)
                    mm(Pf[:, W], r["Hup_l"][:], yh[:, WR], start=False, stop=True)
                # ---- Pl = dl*l2 + 0.5*(y<<1 + y>>1) ----
                mm(Pl[:, W], S("Ddl"), l2[:, W], start=True, stop=False)
                if conv_mode == "fp32":
                    mm(Pl[:, W], S("Hdn05"), y[:, WL], start=False, stop=False)
                    mm(Pl[:, W], S("Hup05"), y[:, WR], start=False, stop=False)
                    mm(Pl[:, W], S("Wl05"), y[:, W], start=False, stop=True)
                else:
                    r = stats_r
                    mm(Pl[:, W], r["Wl05"][:], yh[:, W], start=False, stop=False)
                    mm(Pl[:, W], r["Wl05"][:], yl[:, W], start=False, stop=False)
                    mm(Pl[:, W], r["Hdn05"][:], yh[:, WL], start=False, stop=False)
                    mm(Pl[:, W], r["Hdn05"][:], yl[:, WL], start=False, stop=False)
                    mm(Pl[:, W], r["Hup05"][:], yh[:, WR], start=False, stop=False)
                    mm(Pl[:, W], r["Hup05"][:], yl[:, WR], start=False, stop=True)

                stt = nc.vector.scalar_tensor_tensor
                f_new = state.tile([128, TW], dt.float32, tag="f")
                stt(f_new[:, W], f[:, W], df, Pf[:, W], OP.mult, OP.add)
                u = tmp.tile([128, DW], dt.float32, tag="u")
                stt(u[:], Pl[:, W], 1.0, f_new[:, W], OP.add, OP.mult)
                e2_new = state.tile([128, TW], dt.float32, tag="e2")
                stt(e2_new[:, W], e2[:, W], de, y[:, W], OP.mult, OP.add)
                v = tmp.tile([128, DW], dt.float32, tag="v")
                stt(v[:], e2_new[:, W], -V_E, u[:], OP.mult, OP.add)

                l2_new = state.tile([128, TW], dt.float32, tag="l2")
                nc.scalar.copy(l2_new[:, W], Pl[:, W])

                y_new = ybufs.tile([128, TW], dt.float32, tag="y")
                nc.vector.memset(y_new[:], 0.0)
                v3 = v[:].rearrange("p (s c) -> p s c", s=BPC)[:, :, 0:NBLK]
                y3 = y_new[:, W].rearrange("p (s c) -> p s c", s=BPC)[:, :, 0:NBLK]
                nc.scalar.activation(y3, v3, AF.Sigmoid)
                if conv_mode == "tf32_split":
                    yh_new = ybufs.tile([128, TW], dt.float32r, tag="yh")
                    yl_new = ybufs.tile([128, TW], dt.float32r, tag="yl")
                    nc.vector.tensor_copy(yh_new[:], y_new[:])
                    nc.vector.tensor_tensor(yl_new[:], y_new[:],
                                            yh_new[:].bitcast(dt.float32),
                                            OP.subtract)
                    yh, yl = yh_new, yl_new

                nc.sync.dma_start(yp[t], y_new[:])
                f, e2, l2, y = f_new, e2_new, l2_new, y_new

    _split_sync_waits(nc)
    in_names = ["xp"] + stat_names
    return nc, in_names


def _make_stationaries(w, conv_mode):
    """matmul computes out[i,j] = sum_p W[p,i]*rhs[p,j]; stationary[p, i]
    maps contraction partition p -> output partition i."""
    w0, w1, w2 = [np.float32(v) for v in np.asarray(w, np.float32)]
    i = np.arange(128)
    st = {}

    def banded(a, b, c):
        # out[i] = a*y[i-1] + b*y[i] + c*y[i+1]  (within block)
        Wm = np.zeros((128, 128), np.float32)
        Wm[i, i] = b
        Wm[i[1:] - 1, i[1:]] = a      # W[p=i-1, i] = a
        Wm[i[:-1] + 1, i[:-1]] = c    # W[p=i+1, i] = c
        return Wm

    def halo_dn(val):
        # out[0, j] += val * rhs[127, j]  (rhs = y shifted left one column)
        Wm = np.zeros((128, 128), np.float32)
        Wm[127, 0] = val
        return Wm

    def halo_up(val):
        # out[127, j] += val * rhs[0, j]  (rhs = y shifted right one column)
        Wm = np.zeros((128, 128), np.float32)
        Wm[0, 127] = val
        return Wm

    st["Ident"] = np.eye(128, dtype=np.float32)
    st["Ddl"] = np.eye(128, dtype=np.float32) * np.float32(np.exp(-ALPHA_L))
    if conv_mode == "fp32":
        st["Wc"] = banded(w0, w1, w2)
        st["Hdn"] = halo_dn(w0)
        st["Hup"] = halo_up(w2)
        st["Wl05"] = banded(0.5, 0.0, 0.5)
        st["Hdn05"] = halo_dn(0.5)
        st["Hup05"] = halo_up(0.5)
    else:
        for name, mk, vals in [("Wc", banded, (w0, w1, w2)),
                               ("Hdn", halo_dn, (w0,)),
                               ("Hup", halo_up, (w2,))]:
            Wf = mk(*vals)
            Wh = _round_tf32(Wf)
            st[name] = Wh
            st[name + "_l"] = _round_tf32(Wf - Wh)
        st["Wl05"] = banded(0.5, 0.0, 0.5)   # exact in tf32
        st["Hdn05"] = halo_dn(0.5)
        st["Hup05"] = halo_up(0.5)
    return st


def _pack_x(xc):
    """[BPC, T, L] -> [T, 128, TW] fine-L layout, data window at DO."""
    T_ = xc.shape[1]
    xr = np.ascontiguousarray(
        xc.reshape(BPC, T_, NBLK, 128).transpose(1, 3, 0, 2))  # [T,128,BPC,NBLK]
    out = np.zeros((T_, 128, TW), np.float32)
    g = out[:, :, DO:DO + DW].reshape(T_, 128, BPC, GW)
    g[:, :, :, :NBLK] = xr
    return out


def _unpack_y(ypk, T_):
    """[T, 128, TW] -> [BPC, T, L]"""
    yr = ypk[:, :, DO:DO + DW].reshape(T_, 128, BPC, GW)[:, :, :, :NBLK]
    return np.ascontiguousarray(yr.transpose(2, 0, 3, 1)).reshape(BPC, T_, L)


def run_steps(x, w, n_steps, conv_mode="tf32_split"):
    """Run the kernel for n_steps (full inputs), return [B, n_steps, L]."""
    from concourse.bass_utils import run_bass_kernel_spmd

    key = (n_steps, conv_mode)
    if key not in _CACHE:
        _CACHE[key] = _build_program(n_steps, conv_mode)
    nc, in_names = _CACHE[key]

    st = _make_stationaries(w, conv_mode)
    x = np.asarray(x, np.float32)
    in_maps = []
    for c in range(N_CORES):
        m = {"xp": _pack_x(x[c * BPC:(c + 1) * BPC, :n_steps])}
        m.update(st)
        in_maps.append(m)
    res = run_bass_kernel_spmd(nc, in_maps, list(range(N_CORES)))
    out = np.empty((B, n_steps, L), np.float32)
    for c in range(N_CORES):
        out[c * BPC:(c + 1) * BPC] = _unpack_y(res.results[c]["yp"], n_steps)
    return out


def kernel(x, w):
    return run_steps(x, w, T, conv_mode="tf32_split")


# revision 12
# speedup vs baseline: 1.1309x; 1.1309x over previous
"""Trainium2 Bass kernel for the PCNN recurrence (nn_CCNN1d).

Model (per sample, recurrence over T steps, state vectors of length L):
    f = df*f + x_t + conv3(y, w)          # learned 3-tap conv, zero pad
    l = dl*l + (y shifted left + right)   # fixed [1,0,1] kernel
    u = f * (1 + 0.5*l)
    e = de*e + 10*y
    y = sigmoid(u - e)
outputs y per step.

Sharding: data-parallel over batch B=32 -> 4 samples per NeuronCore x 8.

Per-core layout ("fine-L"): L=8192 split into 64 blocks of 128; partition
p = position within block.  Tiles are [128 x 264]; the matmul data window
is columns [2:262) = 4 sample groups of 65 (64 data blocks + 1 zero pad
column); column 1 is the zero left-halo source of sample 0.  The 3-tap
conv along L becomes one banded 128x128 stationary matmul (within-block
taps) plus two single-element "halo" stationaries applied to rhs views
shifted by one column (cross-block taps); the zero pad columns make
sample boundaries behave like zero padding.  The fp32r (tfloat32) matmul
ISA requires the *dst* AP to be 8-byte aligned with even count, which the
[2:262) window satisfies; rhs offsets are unconstrained so the +-1 column
halo shifts ride on the rhs side.

Per step:
    PE   : Pf(psum) = I@x + Wc@y + Hdn@y(<<1) + Hup@y(>>1)
           Pl(psum) = Ddl@l2 + Wl05@y + Hdn05@y(<<1) + Hup05@y(>>1)
    DVE  : f   = df*f + Pf          (scalar_tensor_tensor)
           u   = (Pl + 1) * f
           e2  = de*e2 + y          (e2 = e/10)
           v   = -10*e2 + u
    ACT  : l2  = copy(Pl)           (next step's Ddl operand)
           y   = sigmoid(v)         (strided write; pads stay 0)

conv_mode="tf32_split": the conv matmuls run as fp32r (tfloat32) pairs
(W = Wh + Wl, y = yh + yl, keeping Wh@yh + Wh@yl + Wl@yh), ~4x faster on
the PE than plain fp32 matmul and equal to fp32 to ~1e-4 absmax here.
conv_mode="fp32": plain fp32 matmuls everywhere (slower, exact).
"""

import numpy as np

B, T, L = 32, 64, 8192
N_CORES = 8
BPC = B // N_CORES          # samples per core
NBLK = L // 128             # 64 blocks per sample
GW = NBLK + 1               # sample group width incl. 1 pad col
DO = 2                      # data window offset (8-byte aligned)
DW = BPC * GW               # data window width = 260
TW = DO + DW + 2            # tile width = 264
ALPHA_F, ALPHA_L, ALPHA_E, V_E = 0.1, 1.0, 1.0, 10.0

_CACHE = {}


def _round_tf32(a):
    a = np.asarray(a, np.float32)
    ai = a.view(np.int32).astype(np.int64)
    return (((ai + 0x1000) & ~0x1FFF).astype(np.int32)).view(np.float32).reshape(a.shape)


def _patch_tile_drain():
    """This toolchain's walrus allows at most one sync wait per instruction;
    spread the TileContext final-drain waits over single-wait nops."""
    import concourse.tile as tile
    from concourse.vector_clock import ScopedClock

    if getattr(tile.TileContext, "_drain_patched", False):
        return

    def _drain_and_barrier(self, tick_clock, wait_clock):
        nc = self.nc
        probe = nc.sync.nop()
        wait_clock.add_sem_waits(probe.ins, ScopedClock({None: tick_clock.global_clock}))
        si = probe.ins.sync_info
        waits = list(si.on_wait) if si and si.on_wait else []
        if len(waits) > 1:
            si.on_wait = waits[:1]
            for w in waits[1:]:
                extra = nc.sync.nop()
                esi = extra.ins.sync_info
                if esi is None:
                    from concourse import mybir
                    extra.ins.sync_info = mybir.SyncInfo(on_wait=[w], on_update=[])
                else:
                    esi.on_wait = [w]
        nc.sync.drain()
        nc.all_engine_barrier()
        assert self.sems is not None
        popped = nc._tile_sem_poison_stack.pop()
        assert popped is self._sem_poison
        nc.clear_and_free_semaphores(list(self.sems.allocated().values()))
        nc.all_engine_barrier()

    tile.TileContext._drain_and_barrier = _drain_and_barrier
    tile.TileContext._drain_patched = True


def _split_sync_waits(nc):
    """Hoist extra sync waits (>1 per instruction) onto same-engine nops
    inserted right before the instruction."""
    from concourse import mybir

    ctr = 0
    for f in nc.m.functions:
        for bb in f.blocks:
            insts = list(bb.instructions)
            if not any(i.sync_info and i.sync_info.on_wait
                       and len(i.sync_info.on_wait) > 1 for i in insts):
                continue
            new_insts = []
            for inst in insts:
                si = inst.sync_info
                waits = list(si.on_wait) if si and si.on_wait else []
                if len(waits) > 1:
                    for w in waits[:-1]:
                        nop = mybir.InstNoOp(name=f"I-wsplit{ctr}", ins=[],
                                             outs=[])
                        ctr += 1
                        nop.engine = inst.engine
                        nop.sync_info = mybir.SyncInfo(on_wait=[w],
                                                       on_update=[])
                        new_insts.append(nop)
                    si.on_wait = [waits[-1]]
                new_insts.append(inst)
            try:
                bb.instructions[:] = new_insts
            except TypeError:
                bb.instructions = new_insts


def _build_program(n_steps, conv_mode):
    """Build the Bass module. Returns (nc, input_names)."""
    _patch_tile_drain()
    from contextlib import ExitStack
    import concourse.bass as bass
    import concourse.tile as tile
    from concourse import mybir

    dt = mybir.dt
    AF = mybir.ActivationFunctionType
    OP = mybir.AluOpType
    df = float(np.float32(np.exp(-ALPHA_F)))
    de = float(np.float32(np.exp(-ALPHA_E)))

    nc = bass.Bass("TRN2", target_bir_lowering=False, debug=False,
                   num_devices=N_CORES)

    xp = nc.dram_tensor("xp", [n_steps, 128, TW], dt.float32,
                        kind="ExternalInput").ap()
    stat_names = ["Wc", "Hdn", "Hup", "Wl05", "Hdn05", "Hup05", "Ddl", "Ident"]
    if conv_mode == "tf32_split":
        stat_names += ["Wc_l", "Hdn_l", "Hup_l"]
    stats_dram = {n: nc.dram_tensor(n, [128, 128], dt.float32,
                                    kind="ExternalInput").ap()
                  for n in stat_names}
    yp = nc.dram_tensor("yp", [n_steps, 128, TW], dt.float32,
                        kind="ExternalOutput").ap()

    W = slice(DO, DO + DW)           # data window [2:262)
    WL = slice(DO - 1, DO + DW - 1)  # rhs shifted left  [1:261)
    WR = slice(DO + 1, DO + DW + 1)  # rhs shifted right [3:263)

    with tile.TileContext(nc) as tc:
        with ExitStack() as ctx:
            const = ctx.enter_context(tc.tile_pool(name="const", bufs=1))
            state = ctx.enter_context(tc.tile_pool(name="state", bufs=2))
            ybufs = ctx.enter_context(tc.tile_pool(name="ybufs", bufs=3))
            xbufs = ctx.enter_context(tc.tile_pool(name="xbufs", bufs=3))
            tmp = ctx.enter_context(tc.tile_pool(name="tmp", bufs=2))
            psum = ctx.enter_context(tc.tile_pool(name="psum", bufs=2,
                                                  space="PSUM"))

            stats = {}
            for n in stat_names:
                st = const.tile([128, 128], dt.float32, tag=f"st_{n}")
                nc.sync.dma_start(st[:], stats_dram[n][:])
                stats[n] = st

            stats_r = {}
            if conv_mode == "tf32_split":
                # fp32r matmul operands need an on-chip rounding producer
                for n in ["Wc", "Hdn", "Hup", "Wl05", "Hdn05", "Hup05",
                          "Wc_l", "Hdn_l", "Hup_l"]:
                    sr = const.tile([128, 128], dt.float32r, tag=f"str_{n}")
                    nc.vector.tensor_copy(sr[:], stats[n][:])
                    stats_r[n] = sr

            def S(n):
                return stats[n][:]

            f = state.tile([128, TW], dt.float32, tag="f")
            l2 = state.tile([128, TW], dt.float32, tag="l2")
            e2 = state.tile([128, TW], dt.float32, tag="e2")
            y = ybufs.tile([128, TW], dt.float32, tag="y")
            nc.vector.memset(f[:], 0.0)
            nc.vector.memset(l2[:], 0.0)
            nc.vector.memset(e2[:], 1.0)
            nc.vector.memset(y[:], 0.0)
            if conv_mode == "tf32_split":
                yh = ybufs.tile([128, TW], dt.float32r, tag="yh")
                yl = ybufs.tile([128, TW], dt.float32r, tag="yl")
                nc.vector.memset(yh[:].bitcast(dt.float32), 0.0)
                nc.vector.memset(yl[:].bitcast(dt.float32), 0.0)

            for t in range(n_steps):
                xt = xbufs.tile([128, TW], dt.float32, tag="x")
                nc.sync.dma_start(xt[:], xp[t])

                Pf = psum.tile([128, TW], dt.float32, tag="Pf")
                Pl = psum.tile([128, TW], dt.float32, tag="Pl")
                mm = nc.tensor.matmul
                # ---- Pf = x + conv3(y, w) ----
                mm(Pf[:, W], S("Ident"), xt[:, W], start=True, stop=False)
                if conv_mode == "fp32":
                    mm(Pf[:, W], S("Wc"), y[:, W], start=False, stop=False)
                    mm(Pf[:, W], S("Hdn"), y[:, WL], start=False, stop=False)
                    mm(Pf[:, W], S("Hup"), y[:, WR], start=False, stop=True)
                else:
                    # all yh-dependent matmuls first; yl arrives later
                    r = stats_r
                    mm(Pf[:, W], r["Wc"][:], yh[:, W], start=False, stop=False)
                    mm(Pf[:, W], r["Wc_l"][:], yh[:, W], start=False, stop=False)
                    mm(Pf[:, W], r["Hdn"][:], yh[:, WL], start=False, stop=False)
                    mm(Pf[:, W], r["Hdn_l"][:], yh[:, WL], start=False, stop=False)
                    mm(Pf[:, W], r["Hup"][:], yh[:, WR], start=False, stop=False)
                    mm(Pf[:, W], r["Hup_l"][:], yh[:, WR], start=False, stop=False)
                    mm(Pf[:, W], r["Wc"][:], yl[:, W], start=False, stop=False)
                    mm(Pf[:, W], r["Hdn"][:], yl[:, WL], start=False, stop=False)
                    mm(Pf[:, W], r["Hup"][:], yl[:, WR], start=False, stop=True)
                # ---- Pl = dl*l2 + 0.5*(y<<1 + y>>1) ----
                mm(Pl[:, W], S("Ddl"), l2[:, W], start=True, stop=False)
                if conv_mode == "fp32":
                    mm(Pl[:, W], S("Hdn05"), y[:, WL], start=False, stop=False)
                    mm(Pl[:, W], S("Hup05"), y[:, WR], start=False, stop=False)
                    mm(Pl[:, W], S("Wl05"), y[:, W], start=False, stop=True)
                else:
                    r = stats_r
                    mm(Pl[:, W], r["Wl05"][:], yh[:, W], start=False, stop=False)
                    mm(Pl[:, W], r["Hdn05"][:], yh[:, WL], start=False, stop=False)
                    mm(Pl[:, W], r["Hup05"][:], yh[:, WR], start=False, stop=False)
                    mm(Pl[:, W], r["Wl05"][:], yl[:, W], start=False, stop=False)
                    mm(Pl[:, W], r["Hdn05"][:], yl[:, WL], start=False, stop=False)
                    mm(Pl[:, W], r["Hup05"][:], yl[:, WR], start=False, stop=True)

                stt = nc.vector.scalar_tensor_tensor
                f_new = state.tile([128, TW], dt.float32, tag="f")
                stt(f_new[:, W], f[:, W], df, Pf[:, W], OP.mult, OP.add)
                u = tmp.tile([128, DW], dt.float32, tag="u")
                stt(u[:], Pl[:, W], 1.0, f_new[:, W], OP.add, OP.mult)
                e2_new = state.tile([128, TW], dt.float32, tag="e2")
                stt(e2_new[:, W], e2[:, W], de, y[:, W], OP.mult, OP.add)
                v = tmp.tile([128, DW], dt.float32, tag="v")
                stt(v[:], e2_new[:, W], -V_E, u[:], OP.mult, OP.add)

                l2_new = state.tile([128, TW], dt.float32, tag="l2")
                nc.scalar.copy(l2_new[:, W], Pl[:, W])

                y_new = ybufs.tile([128, TW], dt.float32, tag="y")
                # dense sigmoid (a strided 3D AP here costs ~4x more on ACT),
                # then re-zero the five pad columns {1,66,131,196,261} that
                # the halo matmuls read as zero padding
                nc.scalar.activation(y_new[:, W], v[:], AF.Sigmoid)
                padsA = (y_new[:, DO - 1:DO - 1 + BPC * GW]
                         .rearrange("p (s c) -> p s c", c=GW)[:, :, 0:1])
                nc.vector.memset(padsA, 0.0)
                nc.vector.memset(y_new[:, DO + DW - 1:DO + DW], 0.0)
                if conv_mode == "tf32_split":
                    yh_new = ybufs.tile([128, TW], dt.float32r, tag="yh")
                    yl_new = ybufs.tile([128, TW], dt.float32r, tag="yl")
                    nc.vector.tensor_copy(yh_new[:], y_new[:])
                    nc.vector.tensor_tensor(yl_new[:], y_new[:],
                                            yh_new[:].bitcast(dt.float32),
                                            OP.subtract)
                    yh, yl = yh_new, yl_new

                nc.sync.dma_start(yp[t], y_new[:])
                f, e2, l2, y = f_new, e2_new, l2_new, y_new

    _split_sync_waits(nc)
    in_names = ["xp"] + stat_names
    return nc, in_names


def _make_stationaries(w, conv_mode):
    """matmul computes out[i,j] = sum_p W[p,i]*rhs[p,j]; stationary[p, i]
    maps contraction partition p -> output partition i."""
    w0, w1, w2 = [np.float32(v) for v in np.asarray(w, np.float32)]
    i = np.arange(128)
    st = {}

    def banded(a, b, c):
        # out[i] = a*y[i-1] + b*y[i] + c*y[i+1]  (within block)
        Wm = np.zeros((128, 128), np.float32)
        Wm[i, i] = b
        Wm[i[1:] - 1, i[1:]] = a      # W[p=i-1, i] = a
        Wm[i[:-1] + 1, i[:-1]] = c    # W[p=i+1, i] = c
        return Wm

    def halo_dn(val):
        # out[0, j] += val * rhs[127, j]  (rhs = y shifted left one column)
        Wm = np.zeros((128, 128), np.float32)
        Wm[127, 0] = val
        return Wm

    def halo_up(val):
        # out[127, j] += val * rhs[0, j]  (rhs = y shifted right one column)
        Wm = np.zeros((128, 128), np.float32)
        Wm[0, 127] = val
        return Wm

    st["Ident"] = np.eye(128, dtype=np.float32)
    st["Ddl"] = np.eye(128, dtype=np.float32) * np.float32(np.exp(-ALPHA_L))
    if conv_mode == "fp32":
        st["Wc"] = banded(w0, w1, w2)
        st["Hdn"] = halo_dn(w0)
        st["Hup"] = halo_up(w2)
        st["Wl05"] = banded(0.5, 0.0, 0.5)
        st["Hdn05"] = halo_dn(0.5)
        st["Hup05"] = halo_up(0.5)
    else:
        for name, mk, vals in [("Wc", banded, (w0, w1, w2)),
                               ("Hdn", halo_dn, (w0,)),
                               ("Hup", halo_up, (w2,))]:
            Wf = mk(*vals)
            Wh = _round_tf32(Wf)
            st[name] = Wh
            st[name + "_l"] = _round_tf32(Wf - Wh)
        st["Wl05"] = banded(0.5, 0.0, 0.5)   # exact in tf32
        st["Hdn05"] = halo_dn(0.5)
        st["Hup05"] = halo_up(0.5)
    return st


def _pack_x(xc):
    """[BPC, T, L] -> [T, 128, TW] fine-L layout, data window at DO."""
    T_ = xc.shape[1]
    xr = np.ascontiguousarray(
        xc.reshape(BPC, T_, NBLK, 128).transpose(1, 3, 0, 2))  # [T,128,BPC,NBLK]
    out = np.zeros((T_, 128, TW), np.float32)
    g = out[:, :, DO:DO + DW].reshape(T_, 128, BPC, GW)
    g[:, :, :, :NBLK] = xr
    return out


def _unpack_y(ypk, T_):
    """[T, 128, TW] -> [BPC, T, L]"""
    yr = ypk[:, :, DO:DO + DW].reshape(T_, 128, BPC, GW)[:, :, :, :NBLK]
    return np.ascontiguousarray(yr.transpose(2, 0, 3, 1)).reshape(BPC, T_, L)


def run_steps(x, w, n_steps, conv_mode="tf32_split"):
    """Run the kernel for n_steps (full inputs), return [B, n_steps, L]."""
    from concourse.bass_utils import run_bass_kernel_spmd

    key = (n_steps, conv_mode)
    if key not in _CACHE:
        _CACHE[key] = _build_program(n_steps, conv_mode)
    nc, in_names = _CACHE[key]

    st = _make_stationaries(w, conv_mode)
    x = np.asarray(x, np.float32)
    in_maps = []
    for c in range(N_CORES):
        m = {"xp": _pack_x(x[c * BPC:(c + 1) * BPC, :n_steps])}
        m.update(st)
        in_maps.append(m)
    res = run_bass_kernel_spmd(nc, in_maps, list(range(N_CORES)))
    out = np.empty((B, n_steps, L), np.float32)
    for c in range(N_CORES):
        out[c * BPC:(c + 1) * BPC] = _unpack_y(res.results[c]["yp"], n_steps)
    return out


def kernel(x, w):
    return run_steps(x, w, T, conv_mode="tf32_split")


# revision 15
# speedup vs baseline: 1.2351x; 1.0921x over previous
"""Trainium2 Bass kernel for the PCNN recurrence (nn_CCNN1d).

Model (per sample, recurrence over T steps, state vectors of length L):
    f = df*f + x_t + conv3(y, w)          # learned 3-tap conv, zero pad
    l = dl*l + (y shifted left + right)   # fixed [1,0,1] kernel
    u = f * (1 + 0.5*l)
    e = de*e + 10*y
    y = sigmoid(u - e)
outputs y per step.

Sharding: data-parallel over batch B=32 -> 4 samples per NeuronCore x 8.

Per-core layout ("fine-L"): L=8192 split into 64 blocks of 128; partition
p = position within block.  Tiles are [128 x 264]; the matmul data window
is columns [2:262) = 4 sample groups of 65 (64 data blocks + 1 zero pad
column); column 1 is the zero left-halo source of sample 0.  The 3-tap
conv along L becomes one banded 128x128 stationary matmul (within-block
taps) plus two single-element "halo" stationaries applied to rhs views
shifted by one column (cross-block taps); the zero pad columns make
sample boundaries behave like zero padding.  The fp32r (tfloat32) matmul
ISA requires the *dst* AP to be 8-byte aligned with even count, which the
[2:262) window satisfies; rhs offsets are unconstrained so the +-1 column
halo shifts ride on the rhs side.

Per step:
    PE   : Pf(psum) = I@x + Wc@y + Hdn@y(<<1) + Hup@y(>>1)
           Pl(psum) = Ddl@l2 + Wl05@y + Hdn05@y(<<1) + Hup05@y(>>1)
    DVE  : f   = df*f + Pf          (scalar_tensor_tensor)
           u   = (Pl + 1) * f
           e2  = de*e2 + y          (e2 = e/10)
           v   = -10*e2 + u
    ACT  : l2  = copy(Pl)           (next step's Ddl operand)
           y   = sigmoid(v)         (strided write; pads stay 0)

conv_mode="tf32_split": the conv matmuls run as fp32r (tfloat32) pairs
(W = Wh + Wl, y = yh + yl, keeping Wh@yh + Wh@yl + Wl@yh), ~4x faster on
the PE than plain fp32 matmul and equal to fp32 to ~1e-4 absmax here.
conv_mode="fp32": plain fp32 matmuls everywhere (slower, exact).
"""

import numpy as np

B, T, L = 32, 64, 8192
N_CORES = 8
BPC = B // N_CORES          # samples per core
NBLK = L // 128             # 64 blocks per sample
GW = NBLK + 1               # sample group width incl. 1 pad col
DO = 2                      # data window offset (8-byte aligned)
DW = BPC * GW               # data window width = 260
TW = DO + DW + 2            # tile width = 264
ALPHA_F, ALPHA_L, ALPHA_E, V_E = 0.1, 1.0, 1.0, 10.0

_CACHE = {}


def _round_tf32(a):
    a = np.asarray(a, np.float32)
    ai = a.view(np.int32).astype(np.int64)
    return (((ai + 0x1000) & ~0x1FFF).astype(np.int32)).view(np.float32).reshape(a.shape)


def _patch_tile_drain():
    """This toolchain's walrus allows at most one sync wait per instruction;
    spread the TileContext final-drain waits over single-wait nops."""
    import concourse.tile as tile
    from concourse.vector_clock import ScopedClock

    if getattr(tile.TileContext, "_drain_patched", False):
        return

    def _drain_and_barrier(self, tick_clock, wait_clock):
        nc = self.nc
        probe = nc.sync.nop()
        wait_clock.add_sem_waits(probe.ins, ScopedClock({None: tick_clock.global_clock}))
        si = probe.ins.sync_info
        waits = list(si.on_wait) if si and si.on_wait else []
        if len(waits) > 1:
            si.on_wait = waits[:1]
            for w in waits[1:]:
                extra = nc.sync.nop()
                esi = extra.ins.sync_info
                if esi is None:
                    from concourse import mybir
                    extra.ins.sync_info = mybir.SyncInfo(on_wait=[w], on_update=[])
                else:
                    esi.on_wait = [w]
        nc.sync.drain()
        nc.all_engine_barrier()
        assert self.sems is not None
        popped = nc._tile_sem_poison_stack.pop()
        assert popped is self._sem_poison
        nc.clear_and_free_semaphores(list(self.sems.allocated().values()))
        nc.all_engine_barrier()

    tile.TileContext._drain_and_barrier = _drain_and_barrier
    tile.TileContext._drain_patched = True


def _split_sync_waits(nc):
    """Hoist extra sync waits (>1 per instruction) onto same-engine nops
    inserted right before the instruction."""
    from concourse import mybir

    ctr = 0
    for f in nc.m.functions:
        for bb in f.blocks:
            insts = list(bb.instructions)
            if not any(i.sync_info and i.sync_info.on_wait
                       and len(i.sync_info.on_wait) > 1 for i in insts):
                continue
            new_insts = []
            for inst in insts:
                si = inst.sync_info
                waits = list(si.on_wait) if si and si.on_wait else []
                if len(waits) > 1:
                    for w in waits[:-1]:
                        nop = mybir.InstNoOp(name=f"I-wsplit{ctr}", ins=[],
                                             outs=[])
                        ctr += 1
                        nop.engine = inst.engine
                        nop.sync_info = mybir.SyncInfo(on_wait=[w],
                                                       on_update=[])
                        new_insts.append(nop)
                    si.on_wait = [waits[-1]]
                new_insts.append(inst)
            try:
                bb.instructions[:] = new_insts
            except TypeError:
                bb.instructions = new_insts


def _build_program(n_steps, conv_mode):
    """Build the Bass module. Returns (nc, input_names)."""
    _patch_tile_drain()
    from contextlib import ExitStack
    import concourse.bass as bass
    import concourse.tile as tile
    from concourse import mybir

    dt = mybir.dt
    AF = mybir.ActivationFunctionType
    OP = mybir.AluOpType
    df = float(np.float32(np.exp(-ALPHA_F)))
    de = float(np.float32(np.exp(-ALPHA_E)))

    nc = bass.Bass("TRN2", target_bir_lowering=False, debug=False,
                   num_devices=N_CORES)

    xp = nc.dram_tensor("xp", [n_steps, 128, TW], dt.float32,
                        kind="ExternalInput").ap()
    stat_names = ["Wc", "Hdn", "Hup", "Wl05", "Hdn05", "Hup05", "Ddl", "Ident"]
    if conv_mode == "tf32_split":
        stat_names += ["Wc_l", "Hdn_l", "Hup_l"]
    stats_dram = {n: nc.dram_tensor(n, [128, 128], dt.float32,
                                    kind="ExternalInput").ap()
                  for n in stat_names}
    yp = nc.dram_tensor("yp", [n_steps, 128, TW], dt.float32,
                        kind="ExternalOutput").ap()

    W = slice(DO, DO + DW)           # data window [2:262)
    WL = slice(DO - 1, DO + DW - 1)  # rhs shifted left  [1:261)
    WR = slice(DO + 1, DO + DW + 1)  # rhs shifted right [3:263)

    with tile.TileContext(nc) as tc:
        with ExitStack() as ctx:
            const = ctx.enter_context(tc.tile_pool(name="const", bufs=1))
            state = ctx.enter_context(tc.tile_pool(name="state", bufs=2))
            ybufs = ctx.enter_context(tc.tile_pool(name="ybufs", bufs=3))
            xbufs = ctx.enter_context(tc.tile_pool(name="xbufs", bufs=3))
            tmp = ctx.enter_context(tc.tile_pool(name="tmp", bufs=2))
            psum = ctx.enter_context(tc.tile_pool(name="psum", bufs=2,
                                                  space="PSUM"))

            stats = {}
            for n in stat_names:
                st = const.tile([128, 128], dt.float32, tag=f"st_{n}")
                nc.sync.dma_start(st[:], stats_dram[n][:])
                stats[n] = st

            stats_r = {}
            if conv_mode == "tf32_split":
                # fp32r matmul operands need an on-chip rounding producer
                for n in ["Wc", "Hdn", "Hup", "Wl05", "Hdn05", "Hup05",
                          "Wc_l", "Hdn_l", "Hup_l"]:
                    sr = const.tile([128, 128], dt.float32r, tag=f"str_{n}")
                    nc.vector.tensor_copy(sr[:], stats[n][:])
                    stats_r[n] = sr

            def S(n):
                return stats[n][:]

            f = state.tile([128, TW], dt.float32, tag="f")
            l2 = state.tile([128, TW], dt.float32, tag="l2")
            e2 = state.tile([128, TW], dt.float32, tag="e2")
            y = ybufs.tile([128, TW], dt.float32, tag="y")
            nc.vector.memset(f[:], 0.0)
            nc.vector.memset(l2[:], 0.0)
            nc.vector.memset(e2[:], 1.0)
            nc.vector.memset(y[:], 0.0)
            if conv_mode == "tf32_split":
                yh = ybufs.tile([128, TW], dt.float32r, tag="yh")
                yl = ybufs.tile([128, TW], dt.float32r, tag="yl")
                nc.vector.memset(yh[:].bitcast(dt.float32), 0.0)
                nc.vector.memset(yl[:].bitcast(dt.float32), 0.0)

            for t in range(n_steps):
                xt = xbufs.tile([128, TW], dt.float32, tag="x")
                nc.sync.dma_start(xt[:], xp[t])

                Pf = psum.tile([128, TW], dt.float32, tag="Pf")
                Pl = psum.tile([128, TW], dt.float32, tag="Pl")
                mm = nc.tensor.matmul
                # ---- Pf = x + conv3(y, w) ----
                mm(Pf[:, W], S("Ident"), xt[:, W], start=True, stop=False)
                if conv_mode == "fp32":
                    mm(Pf[:, W], S("Wc"), y[:, W], start=False, stop=False)
                    mm(Pf[:, W], S("Hdn"), y[:, WL], start=False, stop=False)
                    mm(Pf[:, W], S("Hup"), y[:, WR], start=False, stop=True)
                else:
                    # all yh-dependent matmuls first; yl arrives later
                    r = stats_r
                    mm(Pf[:, W], r["Wc"][:], yh[:, W], start=False, stop=False)
                    mm(Pf[:, W], r["Wc_l"][:], yh[:, W], start=False, stop=False)
                    mm(Pf[:, W], r["Hdn"][:], yh[:, WL], start=False, stop=False)
                    mm(Pf[:, W], r["Hdn_l"][:], yh[:, WL], start=False, stop=False)
                    mm(Pf[:, W], r["Hup"][:], yh[:, WR], start=False, stop=False)
                    mm(Pf[:, W], r["Hup_l"][:], yh[:, WR], start=False, stop=False)
                    mm(Pf[:, W], r["Wc"][:], yl[:, W], start=False, stop=False)
                    mm(Pf[:, W], r["Hdn"][:], yl[:, WL], start=False, stop=False)
                    mm(Pf[:, W], r["Hup"][:], yl[:, WR], start=False, stop=True)
                # ---- Pl = dl*l2 + 0.5*(y<<1 + y>>1) ----
                mm(Pl[:, W], S("Ddl"), l2[:, W], start=True, stop=False)
                if conv_mode == "fp32":
                    mm(Pl[:, W], S("Hdn05"), y[:, WL], start=False, stop=False)
                    mm(Pl[:, W], S("Hup05"), y[:, WR], start=False, stop=False)
                    mm(Pl[:, W], S("Wl05"), y[:, W], start=False, stop=True)
                else:
                    r = stats_r
                    mm(Pl[:, W], r["Wl05"][:], yh[:, W], start=False, stop=False)
                    mm(Pl[:, W], r["Hdn05"][:], yh[:, WL], start=False, stop=False)
                    mm(Pl[:, W], r["Hup05"][:], yh[:, WR], start=False, stop=False)
                    mm(Pl[:, W], r["Wl05"][:], yl[:, W], start=False, stop=False)
                    mm(Pl[:, W], r["Hdn05"][:], yl[:, WL], start=False, stop=False)
                    mm(Pl[:, W], r["Hup05"][:], yl[:, WR], start=False, stop=True)

                stt = nc.vector.scalar_tensor_tensor
                f_new = state.tile([128, TW], dt.float32, tag="f")
                stt(f_new[:, W], f[:, W], df, Pf[:, W], OP.mult, OP.add)
                u = tmp.tile([128, DW], dt.float32, tag="u")
                stt(u[:], Pl[:, W], 1.0, f_new[:, W], OP.add, OP.mult)
                e2_new = state.tile([128, TW], dt.float32, tag="e2")
                stt(e2_new[:, W], e2[:, W], de, y[:, W], OP.mult, OP.add)
                v = tmp.tile([128, DW], dt.float32, tag="v")
                stt(v[:], e2_new[:, W], -V_E, u[:], OP.mult, OP.add)

                l2_new = state.tile([128, TW], dt.float32, tag="l2")
                nc.scalar.copy(l2_new[:, W], Pl[:, W])

                y_new = ybufs.tile([128, TW], dt.float32, tag="y")
                # dense sigmoid (a strided 3D AP here costs ~4x more on ACT),
                # then re-zero the five pad columns {1,66,131,196,261} that
                # the halo matmuls read as zero padding
                nc.scalar.activation(y_new[:, W], v[:], AF.Sigmoid)
                padsA = (y_new[:, DO - 1:DO - 1 + BPC * GW]
                         .rearrange("p (s c) -> p s c", c=GW)[:, :, 0:1])
                nc.vector.memset(padsA, 0.0)
                nc.vector.memset(y_new[:, DO + DW - 1:DO + DW], 0.0)
                if conv_mode == "tf32_split":
                    yh_new = ybufs.tile([128, TW], dt.float32r, tag="yh")
                    yl_new = ybufs.tile([128, TW], dt.float32r, tag="yl")
                    nc.vector.tensor_copy(yh_new[:], y_new[:])
                    nc.vector.tensor_tensor(yl_new[:], y_new[:],
                                            yh_new[:].bitcast(dt.float32),
                                            OP.subtract)
                    yh, yl = yh_new, yl_new

                nc.sync.dma_start(yp[t], y_new[:])
                f, e2, l2, y = f_new, e2_new, l2_new, y_new

    _split_sync_waits(nc)
    in_names = ["xp"] + stat_names
    return nc, in_names


def _make_stationaries(w, conv_mode):
    """matmul computes out[i,j] = sum_p W[p,i]*rhs[p,j]; stationary[p, i]
    maps contraction partition p -> output partition i."""
    w0, w1, w2 = [np.float32(v) for v in np.asarray(w, np.float32)]
    i = np.arange(128)
    st = {}

    def banded(a, b, c):
        # out[i] = a*y[i-1] + b*y[i] + c*y[i+1]  (within block)
        Wm = np.zeros((128, 128), np.float32)
        Wm[i, i] = b
        Wm[i[1:] - 1, i[1:]] = a      # W[p=i-1, i] = a
        Wm[i[:-1] + 1, i[:-1]] = c    # W[p=i+1, i] = c
        return Wm

    def halo_dn(val):
        # out[0, j] += val * rhs[127, j]  (rhs = y shifted left one column)
        Wm = np.zeros((128, 128), np.float32)
        Wm[127, 0] = val
        return Wm

    def halo_up(val):
        # out[127, j] += val * rhs[0, j]  (rhs = y shifted right one column)
        Wm = np.zeros((128, 128), np.float32)
        Wm[0, 127] = val
        return Wm

    st["Ident"] = np.eye(128, dtype=np.float32)
    st["Ddl"] = np.eye(128, dtype=np.float32) * np.float32(np.exp(-ALPHA_L))
    if conv_mode == "fp32":
        st["Wc"] = banded(w0, w1, w2)
        st["Hdn"] = halo_dn(w0)
        st["Hup"] = halo_up(w2)
        st["Wl05"] = banded(0.5, 0.0, 0.5)
        st["Hdn05"] = halo_dn(0.5)
        st["Hup05"] = halo_up(0.5)
    else:
        for name, mk, vals in [("Wc", banded, (w0, w1, w2)),
                               ("Hdn", halo_dn, (w0,)),
                               ("Hup", halo_up, (w2,))]:
            Wf = mk(*vals)
            Wh = _round_tf32(Wf)
            st[name] = Wh
            st[name + "_l"] = _round_tf32(Wf - Wh)
        st["Wl05"] = banded(0.5, 0.0, 0.5)   # exact in tf32
        st["Hdn05"] = halo_dn(0.5)
        st["Hup05"] = halo_up(0.5)
    return st


def _pack_x(xc):
    """[BPC, T, L] -> [T, 128, TW] fine-L layout, data window at DO."""
    T_ = xc.shape[1]
    xr = np.ascontiguousarray(
        xc.reshape(BPC, T_, NBLK, 128).transpose(1, 3, 0, 2))  # [T,128,BPC,NBLK]
    out = np.zeros((T_, 128, TW), np.float32)
    g = out[:, :, DO:DO + DW].reshape(T_, 128, BPC, GW)
    g[:, :, :, :NBLK] = xr
    return out


def _unpack_y(ypk, T_):
    """[T, 128, TW] -> [BPC, T, L]"""
    yr = ypk[:, :, DO:DO + DW].reshape(T_, 128, BPC, GW)[:, :, :, :NBLK]
    return np.ascontiguousarray(yr.transpose(2, 0, 3, 1)).reshape(BPC, T_, L)


def run_steps(x, w, n_steps, conv_mode="tf32_split"):
    """Run the kernel for n_steps (full inputs), return [B, n_steps, L]."""
    from concourse.bass_utils import run_bass_kernel_spmd

    key = (n_steps, conv_mode)
    if key not in _CACHE:
        _CACHE[key] = _build_program(n_steps, conv_mode)
    nc, in_names = _CACHE[key]

    st = _make_stationaries(w, conv_mode)
    x = np.asarray(x, np.float32)
    in_maps = []
    for c in range(N_CORES):
        m = {"xp": _pack_x(x[c * BPC:(c + 1) * BPC, :n_steps])}
        m.update(st)
        in_maps.append(m)
    res = run_bass_kernel_spmd(nc, in_maps, list(range(N_CORES)))
    out = np.empty((B, n_steps, L), np.float32)
    for c in range(N_CORES):
        out[c * BPC:(c + 1) * BPC] = _unpack_y(res.results[c]["yp"], n_steps)
    return out


def kernel(x, w):
    return run_steps(x, w, T, conv_mode="tf32_split")
